# revision 1
# baseline (speedup 1.0000x reference)
"""BinarizedConvNet forward on 8 Trainium2 cores, pure data parallelism.

Per core: 32 images. Convs are computed as PSUM-accumulated matmuls whose
kernel taps are strided free-dim access-pattern offsets (no im2col
materialization). Contraction partitions hold (channel, row-phase) slices;
row-base packing (4x or 2x 32/64-partition groups) keeps the 128x128 PE
array dense via concurrent row/col tile groups. BatchNorm uses exact global
batch statistics via 5 tiny AllReduces of per-channel (sum, sumsq); the
gamma/beta constants ride the same AllReduce buffer pre-divided by 8.

Engine discipline: walrus allows only ONE sync-wait on DVE/ACT instructions,
so every DVE/ACT op is arranged to have at most one unobserved producer
(per-segment single-engine drain+stats pipelines, per-engine scratch and
partial tiles, a DVE observer op after each stats DMA).

Host-side prep: sign() of weights, lhsT layouts, bf16 casts, and the conv1
row-gather xr[img,(c,dy),y,:] = x[img,c,4y+dy,:].
"""

import numpy as np
import ml_dtypes

import concourse.bass as bass
import concourse.mybir as mybir
import concourse.tile as tile
import concourse.tile_sem_assignment as _tsa
from concourse.tile_rust import add_dep_helper
from concourse.bass_utils import run_bass_kernel_spmd

# Keep the SWDGE completion-sem rotation narrow so the kernel-tail drain's
# wait list fits the ISA encoding, and so every lane-FIFO predecessor of the
# small stats DMAs is an already-observed arin/gs DMA (period-2 cadence).
_tsa.NUM_SWDGE_GLOBAL_SEMS = 2

dt = mybir.dt
BF, F32 = dt.bfloat16, dt.float32
AF = mybir.ActivationFunctionType
ALU = mybir.AluOpType
bf16 = ml_dtypes.bfloat16

NCORES = 8
Bc = 32            # images per core
EPS = 1e-5
B = 256

# global batch-stat element counts (full batch, all cores)
N1 = B * 31 * 31
N2 = B * 10 * 10
N3 = B * 8 * 8
N4 = B
N5 = B

# ---------------------------------------------------------------------------
# image bookkeeping (shared between program builder and host prep)
# conv1: base s holds imgs 8s..8s+7 (free slot i0); col-group g = i0 % 4,
# hb1 free slot j = 2s + i0//4  ->  img(g, j) = 8*(j//2) + 4*(j%2) + g
IMGMAP1 = [[8 * (j // 2) + 4 * (j % 2) + g for j in range(8)] for g in range(4)]
# conv2: base b = g; chunk c takes j in [4c,4c+4); hb2 slot t = 4b + (j-4c)
IMGMAP2 = {(c, t): IMGMAP1[t // 4][4 * c + t % 4] for c in range(2) for t in range(16)}
# conv3: base sub = c; chunk c2 takes t in [8c2,8c2+8); hb3 slot u = 8*sub + k
IMGMAP3 = {(c2, u): IMGMAP2[(u // 8, 8 * c2 + u % 8)]
           for c2 in range(2) for u in range(16)}
IMG3_INV = {v: k for k, v in IMGMAP3.items()}          # img -> (c2, u)


def build_program():
    nc = bass.Bass()

    # ---- parameters ----
    xr = nc.declare_dram_parameter("xr", [Bc, 24, 31, 128], BF, isOutput=False)
    bfblob = nc.declare_dram_parameter("bfblob", [128, 1984], BF, isOutput=False)
    fblob = nc.declare_dram_parameter("fblob", [128, 352], F32, isOutput=False)
    wf1 = nc.declare_dram_parameter("wf1", [64, 16384], BF, isOutput=False)
    out_d = nc.declare_dram_parameter("out", [Bc, 9], F32, isOutput=True)

    with tile.TileContext(nc) as tc:
        with (
            tc.tile_pool(name="persist", bufs=1) as pp,
            tc.tile_pool(name="small", bufs=1) as sp,
            tc.tile_pool(name="psum", bufs=7, space="PSUM") as psp,
            tc.tile_pool(name="dram", bufs=1, space="DRAM") as dp,
        ):
            # ---- persistent SBUF tensors ----
            xr_all = pp.tile([128, 8 * 3968], BF, tag="xr_all")
            hb1 = pp.tile([128, 8 * 961], BF, tag="hb1")
            hb2 = pp.tile([128, 16 * 100], BF, tag="hb2")
            hb3 = pp.tile([128, 16 * 64], BF, tag="hb3")
            t_all = pp.tile([64, 32 * 64], BF, tag="t_all")
            bfb = pp.tile([128, 1984], BF, tag="bfb")
            fbl = pp.tile([128, 352], F32, tag="fbl")
            wf1s = pp.tile([64, 16384], BF, tag="wf1s")
            w1s = bfb[:, 0:256]
            w2s = bfb[:, 256:1280]
            w3s = bfb[:, 1280:1856]
            wf2s = bfb[:, 1856:1920]
            idbs = bfb[:, 1920:1984]
            rep32s = fbl[:, 0:128]
            rep64s = fbl[:, 128:256]
            bn1s = fbl[:, 256:258]
            bn2s = fbl[:, 258:260]
            bn3s = fbl[:, 260:262]
            bnf1s = fbl[:, 262:266]
            bnf2s = fbl[0:32, 266:268]
            wf3s = fbl[0:32, 268:277]
            b3s = fbl[0:1, 277:286]
            identf_s = fbl[0:32, 286:318]
            ones_s = fbl[0:1, 318:350]
            # per-engine scratch (avoid cross-engine WAW on scratch)
            scrA = pp.tile([128, 961], BF, tag="scrA")   # ACT
            scrD = pp.tile([128, 961], BF, tag="scrD")   # DVE

            # per-engine stat partials: [sum-cols | sq-cols]
            s1pA = pp.tile([128, 16], F32, tag="s1pA")
            s1pD = pp.tile([128, 16], F32, tag="s1pD")
            s2pA = pp.tile([128, 8], F32, tag="s2pA")
            s2pD = pp.tile([128, 8], F32, tag="s2pD")

            # ---- loads: 7 SWDGE DMAs, each followed later by a Pool
            # absorber so its lane sem is observed by the Pool sequencer ----
            junk = sp.tile([1, 16], F32, tag="junk")
            for s in range(4):
                src = xr[8 * s:8 * (s + 1), :, :, :].rearrange("i k y x -> k i (y x)")
                nc.gpsimd.dma_start(
                    out=xr_all[32 * s:32 * s + 24, :].rearrange(
                        "k (i f) -> k i f", i=8),
                    in_=src)
            nc.gpsimd.dma_start(out=bfb[:, :], in_=bfblob[:, :])
            nc.gpsimd.dma_start(out=fbl[:, :], in_=fblob[:, :])
            nc.gpsimd.dma_start(out=wf1s[:, :], in_=wf1[:, :])
            for s in range(4):
                nc.gpsimd.tensor_copy(junk[0:1, s:s + 1],
                                      xr_all[32 * s:32 * s + 1, 0:2].bitcast(F32))
            nc.gpsimd.tensor_copy(junk[0:1, 4:5], bfb[0:1, 0:2].bitcast(F32))
            nc.gpsimd.tensor_copy(junk[0:1, 5:6], fbl[0:1, 0:1])
            nc.gpsimd.tensor_copy(junk[0:1, 6:7], wf1s[0:1, 0:2].bitcast(F32))


            obsp = psp.tile([128, 16], F32, tag="obs", bufs=1)

            def pe_observe(ap, base=0):
                # Tiny self-contained matmul so the PE absorbs one wait
                # (DMA lane or engine tick) before the real matmuls need it.
                m = min(32, ap.shape[-1])
                nc.tensor.matmul(
                    out=obsp[0:m, 0:1], lhsT=ap[..., 0:m], rhs=ap[..., 0:1],
                    start=True, stop=True, tile_position=(base, 0))

            def drain_copy(eng, dst, src, s1ap, s2ap, scr_):
                """Copy psum->sbuf + S1 accum; then square + S2 accum.
                All three on one engine so each op has <=1 unobserved dep."""
                if eng == 0:
                    nc.vector.tensor_scalar(
                        out=dst, in0=src, scalar1=1.0, scalar2=None,
                        op0=ALU.mult, op1=ALU.add, accum_out=s1ap)
                else:
                    nc.scalar.activation(out=dst, in_=src, func=AF.Copy,
                                         accum_out=s1ap)

            def square_seg(eng, seg, s2ap, width):
                if eng == 0:
                    nc.vector.scalar_tensor_tensor(
                        out=scrD[:, 0:width], in0=seg, scalar=1.0, in1=seg,
                        op0=ALU.mult, op1=ALU.mult, accum_out=s2ap)
                else:
                    nc.scalar.activation(out=scrA[:, 0:width], in_=seg,
                                         func=AF.Square, accum_out=s2ap)

            pe_observe(w1s[0:24, 0:32])
            # =============== conv1 ===============
            # K=24 (c,dy) at base 32s; 8 dx taps accumulate in PSUM;
            # col-group q = i0%4, each accumulation group on its own PSUM
            # bank (concurrent via the PE queue / row+col tiling).
            for s in range(4):
                rview = xr_all[32 * s:32 * s + 24, :].rearrange(
                    "k (i y x) -> k i y x", i=8, y=31, x=128)
                pe_observe(xr_all[32 * s:32 * s + 24, 0:32], base=32 * s)
                for half in range(2):
                    j = 2 * s + half
                    eng = j % 2
                    for yh in range(2):
                        y0, ylen = (0, 16) if yh == 0 else (16, 15)
                        ncols = ylen * 31
                        off = 961 * j + 496 * yh
                        for q in range(4):
                            i0 = 4 * half + q
                            pt = psp.tile([128, 512], F32, tag="ps",
                                          name="c1pt")
                            for dx in range(8):
                                nc.tensor.matmul(
                                    out=pt[32 * q:32 * q + 32, 0:ncols],
                                    lhsT=w1s[32 * s:32 * s + 24,
                                             32 * dx:32 * dx + 32],
                                    rhs=rview[:, i0:i0 + 1, y0:y0 + ylen,
                                              dx:dx + 121:4],
                                    start=(dx == 0), stop=(dx == 7),
                                    tile_position=(32 * s, 32 * q))
                            dst = hb1[32 * q:32 * q + 32, off:off + ncols]
                            src = pt[32 * q:32 * q + 32, 0:ncols]
                            s1p = s1pD if eng == 0 else s1pA
                            cc = 2 * (j // 2) + yh
                            drain_copy(eng, dst, src,
                                       s1p[32 * q:32 * q + 32, cc:cc + 1],
                                       None, None)
                    # sumsq for this whole image column j (after both yh)
                    seg = hb1[:, 961 * j:961 * (j + 1)]
                    s2p = s2pD if eng == 0 else s2pA
                    cj = j // 2
                    square_seg(eng, seg, s2p[:, cj:cj + 1], 961)

            pe_observe(rep32s[0:32, 0:32])
            sc1, sh1 = _bn_finalize_conv(
                nc, sp, psp, dp, s1pA[:, 0:8], s1pD[:, 0:8],
                s2pA[:, 0:4], s2pD[:, 0:4], rep32s, 32, 4, bn1s, N1, "ar1", junk, 8, pe_observe)
            nc.scalar.activation(out=hb1[:, :], in_=hb1[:, :], func=AF.Relu,
                                 scale=sc1[:, 0:1], bias=sh1[:, 0:1])

            # =============== conv2 ===============
            # K=32 (c2) at base 32b; 16 taps; col-group = img chunk c
            pe_observe(sh1[0:128, 0:1])
            pe_observe(w2s[0:32, 0:32])
            for b in range(4):
                hview = hb1[32 * b:32 * b + 32, :].rearrange(
                    "k (j y x) -> k j y x", j=8, y=31, x=31)
                eng = b % 2
                for c in range(2):
                    pt = psp.tile([128, 512], F32, tag="ps", name="c2pt")
                    first = True
                    for dy in range(4):
                        for dx in range(4):
                            nc.tensor.matmul(
                                out=pt[64 * c:64 * c + 64, 0:400],
                                lhsT=w2s[32 * b:32 * b + 32,
                                         64 * (4 * dy + dx):64 * (4 * dy + dx) + 64],
                                rhs=hview[:, 4 * c:4 * c + 4,
                                          dy:dy + 28:3, dx:dx + 28:3],
                                start=first, stop=(dy == 3 and dx == 3),
                                tile_position=(32 * b, 64 * c))
                            first = False
                    dst = hb2[64 * c:64 * c + 64, 400 * b:400 * b + 400]
                    src = pt[64 * c:64 * c + 64, 0:400]
                    s1p = s1pD if eng == 0 else s1pA
                    cc = 8 + b // 2
                    drain_copy(eng, dst, src,
                               s1p[64 * c:64 * c + 64, cc:cc + 1], None, None)
                seg = hb2[:, 400 * b:400 * (b + 1)]
                s2p = s2pD if eng == 0 else s2pA
                cj = 4 + b // 2
                square_seg(eng, seg, s2p[:, cj:cj + 1], 400)

            pe_observe(rep64s[0:64, 0:32])
            sc2, sh2 = _bn_finalize_conv(
                nc, sp, psp, dp, s1pA[:, 8:10], s1pD[:, 8:10],
                s2pA[:, 4:6], s2pD[:, 4:6], rep64s, 64, 2, bn2s, N2, "ar2", junk, 9, pe_observe)
            nc.scalar.activation(out=hb2[:, :], in_=hb2[:, :], func=AF.Relu,
                                 scale=sc2[:, 0:1], bias=sh2[:, 0:1])

            # =============== conv3 ===============
            # K=64 (c3) at base 64*sub; 9 taps; col-group = img chunk c2
            pe_observe(sh2[0:128, 0:1])
            pe_observe(w3s[0:64, 0:32])
            for sub in range(2):
                hview = hb2[64 * sub:64 * sub + 64, :].rearrange(
                    "k (t y x) -> k t y x", t=16, y=10, x=10)
                eng = sub % 2
                for c2 in range(2):
                    pt = psp.tile([128, 512], F32, tag="ps", name="c3pt")
                    first = True
                    for dy in range(3):
                        for dx in range(3):
                            nc.tensor.matmul(
                                out=pt[64 * c2:64 * c2 + 64, 0:512],
                                lhsT=w3s[64 * sub:64 * sub + 64,
                                         64 * (3 * dy + dx):64 * (3 * dy + dx) + 64],
                                rhs=hview[:, 8 * c2:8 * c2 + 8,
                                          dy:dy + 8, dx:dx + 8],
                                start=first, stop=(dy == 2 and dx == 2),
                                tile_position=(64 * sub, 64 * c2))
                            first = False
                    dst = hb3[64 * c2:64 * c2 + 64, 512 * sub:512 * sub + 512]
                    src = pt[64 * c2:64 * c2 + 64, 0:512]
                    s1p = s1pD if eng == 0 else s1pA
                    cc = 12
                    drain_copy(eng, dst, src,
                               s1p[64 * c2:64 * c2 + 64, cc:cc + 1], None, None)
                seg = hb3[:, 512 * sub:512 * (sub + 1)]
                s2p = s2pD if eng == 0 else s2pA
                square_seg(eng, seg, s2p[:, 6:7], 512)

            sc3, sh3 = _bn_finalize_conv(
                nc, sp, psp, dp, s1pA[:, 12:13], s1pD[:, 12:13],
                s2pA[:, 6:7], s2pD[:, 6:7], rep64s, 64, 2, bn3s, N3, "ar3", junk, 10, pe_observe)
            nc.scalar.activation(out=hb3[:, :], in_=hb3[:, :], func=AF.Relu,
                                 scale=sc3[:, 0:1], bias=sh3[:, 0:1])

            # =============== fc1 ===============
            # transpose each image's [64 oc, 64 pos] block -> t_all[pos, img*64+oc]
            pe_observe(sh3[0:128, 0:1])
            pe_observe(idbs[0:64, 0:32])
            for v in range(Bc):
                c2, u = IMG3_INV[v]
                tp = psp.tile([64, 64], BF, tag="ps", name="tp")
                nc.tensor.transpose(
                    out=tp[0:64, 0:64],
                    in_=hb3[64 * c2:64 * c2 + 64, 64 * u:64 * u + 64],
                    identity=idbs[64 * c2:64 * c2 + 64, 0:64])
                nc.scalar.copy(out=t_all[0:64, 64 * v:64 * v + 64],
                               in_=tp[0:64, 0:64])

            pe_observe(wf1s[0:64, 0:32])
            tview = t_all[0:64, :].rearrange("p (v o) -> p o v", o=64)
            pf1s = sp.tile([128, 256], F32, tag="pf1s")
            for g in range(4):
                ptf1 = psp.tile([128, 256], F32, tag="ps", name="f1pt")
                for k in range(16):
                    oc = 16 * g + k
                    nc.tensor.matmul(
                        out=ptf1[32 * g:32 * g + 32, 0:256],
                        lhsT=tview[:, oc:oc + 1, :],
                        rhs=wf1s[:, 256 * oc:256 * oc + 256],
                        start=(k == 0), stop=(k == 15),
                        tile_position=(0, 32 * g))
                nc.scalar.copy(out=pf1s[32 * g:32 * g + 32, :],
                               in_=ptf1[32 * g:32 * g + 32, 0:256])
            ptf1b = psp.tile([128, 512], F32, tag="ps", name="f1ptb")
            nc.tensor.matmul(out=ptf1b[0:32, 0:256], lhsT=rep32s[:, 0:32],
                             rhs=pf1s[:, :], start=True, stop=True)
            z4s = sp.tile([32, 256], F32, tag="z4s")
            nc.scalar.copy(out=z4s[:, :], in_=ptf1b[0:32, 0:256])

            pe_observe(identf_s[:, 0:16])
            # transpose z4 -> [256 feat, 32 img] halves; stats per feature
            zt4 = [sp.tile([128, 32], F32, tag=f"zt4_{h}", name=f"zt4_{h}")
                   for h in range(2)]
            stf = sp.tile([128, 8], F32, tag="stf")
            for h in range(2):
                ztp = psp.tile([128, 32], F32, tag="ps", name="ztp")
                nc.tensor.transpose(out=ztp[0:128, 0:32],
                                    in_=z4s[0:32, 128 * h:128 * h + 128],
                                    identity=identf_s[:, 0:32])
                nc.vector.tensor_copy(zt4[h][:, :], ztp[0:128, 0:32])
                nc.scalar.activation(
                    out=scrA[:, 0:32], in_=ztp[0:128, 0:32], func=AF.Copy,
                    accum_out=stf[:, 4 * h:4 * h + 1])
                nc.scalar.activation(
                    out=scrA[:, 32:64], in_=ztp[0:128, 0:32], func=AF.Square,
                    accum_out=stf[:, 4 * h + 1:4 * h + 2])
                nc.scalar.copy(out=stf[:, 4 * h + 2:4 * h + 4],
                               in_=bnf1s[:, 2 * h:2 * h + 2])

            arin4 = dp.tile([128, 8], F32, tag="ar4i")
            arout4 = dp.tile([128, 8], F32, tag="ar4o")
            nc.gpsimd.dma_start(out=arin4[:, :], in_=stf[:, :])
            nc.gpsimd.collective_compute(
                "AllReduce", ALU.add, replica_groups=[list(range(NCORES))],
                ins=[arin4.opt()], outs=[arout4.opt()])
            gsf = sp.tile([128, 8], F32, tag="gsf")
            nc.gpsimd.dma_start(out=gsf[:, :], in_=arout4[:, :])
            nc.gpsimd.tensor_copy(junk[0:1, 7:8], gsf[0:1, 0:1])
            h4 = [sp.tile([128, 32], BF, tag=f"h4_{h}", name=f"h4_{h}")
                  for h in range(2)]
            sh4s = []
            for h in range(2):
                sc, sh, _ = _bn_scale_shift(nc, sp, gsf[:, 4 * h:4 * h + 4],
                                            N4, 128, f"f1{h}")
                sh4s.append(sh)
                nc.scalar.activation(out=h4[h][:, :], in_=zt4[h][:, :],
                                     func=AF.Relu, scale=sc[:, 0:1],
                                     bias=sh[:, 0:1])

            # =============== fc2 ===============
            pe_observe(sh4s[0][0:128, 0:1])
            pe_observe(sh4s[1][0:128, 0:1])
            pe_observe(wf2s[0:128, 0:32])
            ptf2 = psp.tile([128, 512], F32, tag="ps", name="f2pt")
            for h in range(2):
                nc.tensor.matmul(out=ptf2[0:32, 0:32],
                                 lhsT=wf2s[:, 32 * h:32 * h + 32],
                                 rhs=h4[h][:, :],
                                 start=(h == 0), stop=(h == 1))
            z5s = sp.tile([32, 32], F32, tag="z5s")
            stf2 = sp.tile([32, 4], F32, tag="stf2")
            nc.vector.tensor_copy(z5s[:, :], ptf2[0:32, 0:32])
            nc.scalar.activation(
                out=scrA[0:32, 0:32], in_=ptf2[0:32, 0:32], func=AF.Copy,
                accum_out=stf2[:, 0:1])
            nc.scalar.activation(
                out=scrA[0:32, 32:64], in_=ptf2[0:32, 0:32], func=AF.Square,
                accum_out=stf2[:, 1:2])
            nc.scalar.copy(out=stf2[:, 2:4], in_=bnf2s[:, :])
            arin5 = dp.tile([32, 4], F32, tag="ar5i")
            arout5 = dp.tile([32, 4], F32, tag="ar5o")
            nc.gpsimd.dma_start(out=arin5[:, :], in_=stf2[:, :])
            coll5 = nc.gpsimd.collective_compute(
                "AllReduce", ALU.add, replica_groups=[list(range(NCORES))],
                ins=[arin5.opt()], outs=[arout5.opt()])
            gs5 = sp.tile([32, 4], F32, tag="gs5")
            gs5dma = nc.gpsimd.dma_start(out=gs5[:, :], in_=arout5[:, :])
            pool_last = nc.gpsimd.tensor_copy(junk[0:1, 11:12], gs5[0:1, 0:1])
            sc5, sh5, dve_last = _bn_scale_shift(nc, sp, gs5, N5, 32, "f2")
            h5 = sp.tile([32, 32], F32, tag="h5")
            nc.scalar.activation(out=h5[:, :], in_=z5s[:, :], func=AF.Relu,
                                 scale=sc5[:, 0:1], bias=sh5[:, 0:1])

            # =============== fc3 ===============
            pe_observe(sh5[0:32, 0:1])
            pe_observe(wf3s[0:32, 0:4])
            pe_observe(ones_s[0:1, 0:16])
            pe_observe(b3s[0:1, 0:4])
            ptf3 = psp.tile([128, 512], F32, tag="ps", name="f3pt")
            nc.tensor.matmul(out=ptf3[0:32, 0:9], lhsT=h5[:, :], rhs=wf3s[:, :],
                             start=True, stop=False)
            pe_last = nc.tensor.matmul(out=ptf3[0:32, 0:9],
                                       lhsT=ones_s[0:1, :],
                                       rhs=b3s[0:1, :], start=False, stop=True)
            outs = sp.tile([32, 9], F32, tag="outs")
            act_last = nc.scalar.copy(out=outs[:, :], in_=ptf3[0:32, 0:9])
            out_dma = nc.gpsimd.dma_start(out=out_d[:, :], in_=outs[:, :])

            # chained single-wait SP drains so the Tile tail drain's wait
            # list fits the ISA encoding: SP directly observes every proc's
            # final tick, one per drain.
            for i, dep in enumerate([coll5, gs5dma, pool_last, out_dma,
                                     act_last, dve_last, pe_last]):
                dr = nc.sync.drain(fusable=False)
                add_dep_helper(dr.ins, dep.ins, reason=f"tail-funnel-{i}")

    return nc


def _bn_scale_shift(nc, sp, gs, n, parts, name):
    """gs [parts,4] = (S1, S2, gamma, beta) landed by a single DMA.
    Chain arranged so every DVE/ACT op has <=1 unobserved producer."""
    m = sp.tile([parts, 1], F32, tag=f"m_{name}", name=f"m_{name}")
    q = sp.tile([parts, 1], F32, tag=f"q_{name}", name=f"q_{name}")
    msq = sp.tile([parts, 1], F32, tag=f"ms_{name}", name=f"ms_{name}")
    var = sp.tile([parts, 1], F32, tag=f"v_{name}", name=f"v_{name}")
    sd = sp.tile([parts, 1], F32, tag=f"sd_{name}", name=f"sd_{name}")
    rsd = sp.tile([parts, 1], F32, tag=f"rs_{name}", name=f"rs_{name}")
    sc = sp.tile([parts, 1], F32, tag=f"sc_{name}", name=f"sc_{name}")
    tmp = sp.tile([parts, 1], F32, tag=f"tp_{name}", name=f"tp_{name}")
    shf = sp.tile([parts, 1], F32, tag=f"sh_{name}", name=f"sh_{name}")
    obs = sp.tile([parts, 4], F32, tag=f"ob_{name}", name=f"ob_{name}")
    # DVE observer of the gs DMA so later DVE reads of gs need no new wait
    nc.vector.tensor_copy(obs[:, :], gs[:, :])
    nc.scalar.mul(out=m[:, :], in_=gs[:, 0:1], mul=1.0 / n)       # ACT
    nc.scalar.mul(out=q[:, :], in_=gs[:, 1:2], mul=1.0 / n)       # ACT
    nc.scalar.square(out=msq[:, :], in_=m[:, :])                  # ACT
    nc.vector.tensor_sub(out=var[:, :], in0=q[:, :], in1=msq[:, :])   # DVE
    nc.vector.tensor_scalar_add(out=var[:, :], in0=var[:, :], scalar1=EPS)
    nc.scalar.sqrt(out=sd[:, :], in_=var[:, :])                   # ACT
    nc.vector.reciprocal(out=rsd[:, :], in_=sd[:, :])             # DVE
    nc.vector.tensor_mul(out=sc[:, :], in0=rsd[:, :], in1=gs[:, 2:3])
    nc.vector.tensor_mul(out=tmp[:, :], in0=m[:, :], in1=sc[:, :])
    last = nc.vector.tensor_sub(out=shf[:, :], in0=gs[:, 3:4], in1=tmp[:, :])
    return sc, shf, last


def _bn_finalize_conv(nc, sp, psp, dp, s1a, s1d, s2a, s2d, reps, C, nrep,
                      bnd, n, name, junk_g=None, jcol=0, obs=None):
    """Reduce per-engine partials on DVE, indicator-matmul to per-channel,
    AllReduce (with gamma/beta riding along), then scale/shift."""
    t1 = sp.tile([128, 1], F32, tag=f"t1_{name}", name=f"t1_{name}")
    t2 = sp.tile([128, 1], F32, tag=f"t2_{name}", name=f"t2_{name}")
    ss = sp.tile([128, 2], F32, tag=f"ss_{name}", name=f"ss_{name}")
    # PE observer of the ACT-written partials: absorbs the psum-slot WAR
    # tick so the indicator matmul carries only the DVE wait.
    obs(s1a)
    nc.vector.tensor_reduce(out=t1[:, :], in_=s1a,
                            axis=mybir.AxisListType.X, op=ALU.add)
    nc.vector.tensor_reduce(out=t2[:, :], in_=s1d,
                            axis=mybir.AxisListType.X, op=ALU.add)
    nc.vector.tensor_add(out=ss[:, 0:1], in0=t1[:, :], in1=t2[:, :])
    nc.vector.tensor_reduce(out=t1[:, :], in_=s2a,
                            axis=mybir.AxisListType.X, op=ALU.add)
    nc.vector.tensor_reduce(out=t2[:, :], in_=s2d,
                            axis=mybir.AxisListType.X, op=ALU.add)
    nc.vector.tensor_add(out=ss[:, 1:2], in0=t1[:, :], in1=t2[:, :])
    pt = psp.tile([128, 512], F32, tag="ps", name=f"bnpt_{name}")
    nc.tensor.matmul(out=pt[0:128, 0:2], lhsT=reps[:, 0:128], rhs=ss[:, :],
                     start=True, stop=True)
    st = sp.tile([128, 4], F32, tag=f"st_{name}", name=f"st_{name}")
    nc.scalar.copy(out=st[:, 0:2], in_=pt[0:128, 0:2])
    nc.scalar.copy(out=st[:, 2:4], in_=bnd[:, :])
    arin = dp.tile([128, 4], F32, tag=f"{name}i", name=f"{name}i")
    arout = dp.tile([128, 4], F32, tag=f"{name}o", name=f"{name}o")
    nc.gpsimd.dma_start(out=arin[:, :], in_=st[:, :])
    nc.gpsimd.collective_compute(
        "AllReduce", ALU.add, replica_groups=[list(range(NCORES))],
        ins=[arin.opt()], outs=[arout.opt()])
    gs = sp.tile([128, 4], F32, tag=f"gs_{name}", name=f"gs_{name}")
    nc.gpsimd.dma_start(out=gs[:, :], in_=arout[:, :])
    nc.gpsimd.tensor_copy(junk_g[0:1, jcol:jcol + 1], gs[0:1, 0:1])
    sc, shf, _ = _bn_scale_shift(nc, sp, gs, n, 128, name)
    return sc, shf


# ---------------------------------------------------------------------------
# host-side prep
def _prep_consts(conv1_w, conv2_w, conv3_w, fc1_w, fc2_w, fc3_w, fc3_b,
                 bn1_g, bn1_b, bn2_g, bn2_b, bn3_g, bn3_b,
                 bnf1_g, bnf1_b, bnf2_g, bnf2_b):
    sgn = lambda w: np.sign(np.asarray(w)).astype(np.float32)
    w1, w2, w3 = sgn(conv1_w), sgn(conv2_w), sgn(conv3_w)
    wf1_, wf2_ = sgn(fc1_w), sgn(fc2_w)

    w1r = np.zeros((128, 256), np.float32)
    blk = w1.transpose(1, 2, 3, 0).reshape(24, 8, 32)   # (c,dy), dx, oc
    for s in range(4):
        w1r[32 * s:32 * s + 24] = blk.reshape(24, 256)

    w2r = np.zeros((128, 1024), np.float32)
    blk2 = w2.transpose(1, 2, 3, 0).reshape(32, 16, 64)  # c2, (dy,dx), oc
    for b in range(4):
        w2r[32 * b:32 * b + 32] = blk2.reshape(32, 1024)

    w3r = np.zeros((128, 576), np.float32)
    blk3 = w3.transpose(1, 2, 3, 0).reshape(64, 9, 64)
    for s in range(2):
        w3r[64 * s:64 * s + 64] = blk3.reshape(64, 576)

    # wf1 [64 yx, 64 oc * 256 of]: wf1s[yx, oc*256+of] = sign(fc1_w[of, oc*64+yx])
    wf1r = np.ascontiguousarray(
        wf1_.reshape(256, 64, 64).transpose(2, 1, 0).reshape(64, 64 * 256))
    # wf2 [128 fs, (h, of)]
    wf2r = np.ascontiguousarray(
        wf2_.reshape(32, 2, 128).transpose(2, 1, 0).reshape(128, 64))
    wf3r = np.ascontiguousarray(np.asarray(fc3_w).astype(np.float32).T)  # [32, 9]
    b3r = np.asarray(fc3_b).astype(np.float32).reshape(1, 9)

    # bn consts pre-divided by NCORES: they are summed by the AllReduce
    rep = lambda g, b, nr: np.tile(
        np.stack([np.asarray(g), np.asarray(b)], axis=1).astype(np.float32),
        (nr, 1)) / NCORES
    bfblob = np.zeros((128, 1984), np.float32)
    bfblob[:, 0:256] = w1r
    bfblob[:, 256:1280] = w2r
    bfblob[:, 1280:1856] = w3r
    bfblob[:, 1856:1920] = wf2r
    bfblob[:, 1920:1984] = np.tile(np.eye(64, dtype=np.float32), (2, 1))

    fblob = np.zeros((128, 352), np.float32)
    fblob[:, 0:128] = np.tile(np.eye(32, dtype=np.float32), (4, 4))
    fblob[:, 128:256] = np.tile(np.eye(64, dtype=np.float32), (2, 2))
    fblob[:, 256:258] = rep(bn1_g, bn1_b, 4)
    fblob[:, 258:260] = rep(bn2_g, bn2_b, 2)
    fblob[:, 260:262] = rep(bn3_g, bn3_b, 2)
    fblob[:, 262:266] = np.concatenate([
        np.stack([np.asarray(bnf1_g)[:128], np.asarray(bnf1_b)[:128]], 1),
        np.stack([np.asarray(bnf1_g)[128:], np.asarray(bnf1_b)[128:]], 1)],
        axis=1).astype(np.float32) / NCORES
    fblob[0:32, 266:268] = rep(bnf2_g, bnf2_b, 1)
    fblob[0:32, 268:277] = wf3r
    fblob[0:1, 277:286] = b3r
    fblob[0:32, 286:318] = np.eye(32, dtype=np.float32)
    fblob[0:1, 318:350] = 1.0
    consts = {
        "bfblob": bfblob.astype(bf16),
        "fblob": fblob,
        "wf1": wf1r.astype(bf16),
    }
    return consts


def _prep_xr(xc):
    """[Bc,3,128,128] f32 -> [Bc,24,31,128] bf16 row-gather."""
    out = np.empty((xc.shape[0], 24, 31, 128), dtype=bf16)
    for c in range(3):
        for dy in range(8):
            out[:, c * 8 + dy] = xc[:, c, dy:dy + 121:4, :].astype(bf16)
    return out


_NC_CACHE = None


def kernel(**inputs):
    global _NC_CACHE
    if _NC_CACHE is None:
        _NC_CACHE = build_program()
    nc = _NC_CACHE

    x = np.asarray(inputs["x"])
    consts = _prep_consts(
        inputs["conv1_w"], inputs["conv2_w"], inputs["conv3_w"],
        inputs["fc1_w"], inputs["fc2_w"], inputs["fc3_w"], inputs["fc3_b"],
        inputs["bn1_g"], inputs["bn1_b"], inputs["bn2_g"], inputs["bn2_b"],
        inputs["bn3_g"], inputs["bn3_b"],
        inputs["bnf1_g"], inputs["bnf1_b"], inputs["bnf2_g"], inputs["bnf2_b"])

    in_maps = []
    for i in range(NCORES):
        m = dict(consts)
        m["xr"] = _prep_xr(x[Bc * i:Bc * (i + 1)])
        in_maps.append(m)

    res = run_bass_kernel_spmd(nc, in_maps, list(range(NCORES)))
    out = np.concatenate([res.results[i]["out"] for i in range(NCORES)], axis=0)
    return out.astype(np.float32)


if __name__ == "__main__":
    nc = build_program()
    print("program built ok")

